# revision 46
# baseline (speedup 1.0000x reference)
"""Multi-head attention Trainium2 kernel (8 NeuronCores, tensor-parallel over heads).

Strategy:
  - 16 heads / 8 cores = 2 heads per core. x is replicated; Wq/Wk/Wv sharded by
    head; Wp row-sharded (contraction dim). Each core computes a partial
    projection output [B*T, D]; the host sums the 8 partials (+bias).
  - On chip, all contractions need the contracted dim on SBUF partitions, so the
    host passes xT = x.reshape(BT, D).T and per-core transposed weight slices.
  - qT/kT are computed packed [128 = 2 heads x 64, BT]. Scores are computed
    transposed (s on partitions, t on free) so softmax normalization can ride
    the attn@v matmul: lhsT = [v_h | ones] gives unnormalized out^T plus the
    softmax denominator Z as an extra row. Head 0 lands at PSUM partitions
    0..64 (Z at 64), head 1 at partitions 63..128 (Z at 63, ones column FIRST
    in its vaug slice) so both normalization multiplies are partition-aligned
    with their outT rows -- no cross-partition staging DMA.
  - Causality: only lower-triangular blocks are computed; matmuls straddling
    the diagonal are column-trimmed to the causal region and share one
    [128,128] staircase mask (applied multiplicatively after exp).
  - cfg "tf32": every matmul operand tile/DRAM tensor is float32r (TF32-like,
    1 cycle/row vs 4 for fp32 when N>=256). The BIR verifier requires f32r
    matmul inputs be *produced* as f32r, so DRAM inputs are declared f32r and
    every on-chip producer (DVE/ACT/Pool copies, exp, muls) writes f32r.
  - Emission is interleaved: engines execute their streams in program order,
    so QKV (next block) and projection (previous block) PE work is emitted in
    small units between attention i-steps, filling PE stalls that the
    scores->exp->attn@v dependency chain would otherwise expose.
"""

import numpy as np

B, T, D, H, HD = 2, 2048, 1024, 16, 64
NCORES = 8
HPC = H // NCORES          # heads per core = 2
CH = HPC * HD              # channels per core = 128
BT = B * T

_CACHE = {}


def _build(b, t, d, cfg):
    """Build + compile the per-core Bass program."""
    import concourse.tile as tile
    from concourse import bacc, mybir
    from concourse.masks import make_identity
    from contextlib import ExitStack

    f32 = mybir.dt.float32
    mdt = mybir.dt.float32r if cfg == "tf32" else f32

    bt = b * t
    KT = d // 128            # k-tiles over the model dim
    TBLK = min(512, t)       # t-block width for scores/attn
    NJ = t // TBLK           # t-blocks per batch
    NSB = bt // 128          # 128-row s-blocks over B*T
    SPT = TBLK // 128        # s-blocks per t-block

    nc = bacc.Bacc("TRN2", target_bir_lowering=False, debug=False)

    xT = nc.dram_tensor("xT", [d, bt], mdt, kind="ExternalInput").ap()
    wq = nc.dram_tensor("wq", [d, CH], mdt, kind="ExternalInput").ap()
    wk = nc.dram_tensor("wk", [d, CH], mdt, kind="ExternalInput").ap()
    wv = nc.dram_tensor("wv", [d, CH], mdt, kind="ExternalInput").ap()
    wp = nc.dram_tensor("wp", [CH, d], mdt, kind="ExternalInput").ap()
    bf16 = mybir.dt.bfloat16
    out_p = nc.dram_tensor("out_p", [bt, d], bf16, kind="ExternalOutput").ap()

    with tile.TileContext(nc) as tc, ExitStack() as top:
        persist = top.enter_context(tc.tile_pool(name="persist", bufs=1))

        # ---- persistent tiles ----
        qT_sb = persist.tile([128, bt], mdt, tag="qT")
        kT_sb = persist.tile([128, bt], mdt, tag="kT")
        # per 128-row s-block: [v_h0 | 1 | pad | v_h1 | 1 | pad]
        vaug = persist.tile([128, NSB, 66 * HPC], mdt, tag="vaug")
        outT_sb = persist.tile([128, bt], mdt, tag="outT")
        wq_3d = persist.tile([128, KT, CH], mdt, tag="wq")
        wk_3d = persist.tile([128, KT, CH], mdt, tag="wk")
        wv_3d = persist.tile([128, KT, CH], mdt, tag="wv")
        wq_sb = [wq_3d[:, kt, :] for kt in range(KT)]
        wk_sb = [wk_3d[:, kt, :] for kt in range(KT)]
        wv_sb = [wv_3d[:, kt, :] for kt in range(KT)]
        wp_sb = persist.tile([128, d], mdt, tag="wp")
        # copy of wp rows 64:128 at partitions 0:64 for the tail's split-K
        # projection (h1 output stays at partitions 0:64 there)
        wp2_sb = persist.tile([HD, d], mdt, tag="wp2")
        ident_f32 = persist.tile([128, 128], f32, tag="ident_f32")
        mask = persist.tile([128, 128], f32, tag="mask")
        ones_f32 = persist.tile([128, max(NSB, HD)], f32, tag="ones_f32")
        ones1 = persist.tile([65, HD], mdt, tag="ones1")

        # weight loads spread across dispatch queues so SP stays free for xt
        # wq first on SP so its transfer beats the xt loads to the DMA
        # engines; wk/wv/wp ride Pool's SWDGE (cheap dispatch)
        nc.sync.dma_start(
            out=wq_3d[:], in_=wq.rearrange("(kt p) m -> p kt m", p=128))
        nc.gpsimd.dma_start(
            out=wk_3d[:], in_=wk.rearrange("(kt p) m -> p kt m", p=128))
        nc.gpsimd.dma_start(
            out=wv_3d[:], in_=wv.rearrange("(kt p) m -> p kt m", p=128))

        make_identity(nc, ident_f32[:])
        if mdt != f32:
            ident = persist.tile([128, 128], mdt, tag="ident")
            nc.scalar.copy(ident[:], ident_f32[:])
        else:
            ident = ident_f32
        nc.gpsimd.memset(ones_f32[:], 1.0)
        # ones row for the 1/Z broadcast matmuls (K=1 lhsT at partition 64)
        nc.scalar.copy(ones1[64:65, :], ones_f32[64:65, 0:HD])
        # ones columns of vaug (softmax denominator rides the attn@v matmul)
        for g in range(HPC):
            nc.scalar.copy(
                vaug[:, :, g * 66 + 64:g * 66 + 65],
                ones_f32[:, 0:NSB].rearrange("p (n o) -> p n o", o=1),
            )
        # staircase mask: keep (p <= c), i.e. upper-triangular incl. diagonal
        nc.gpsimd.memset(mask[:], 1.0)
        nc.gpsimd.affine_select(
            out=mask[:], in_=mask[:],
            compare_op=mybir.AluOpType.is_ge,
            fill=0.0, base=0,
            # iota = -p + c ; keep when >= 0
            pattern=[[1, 128]], channel_multiplier=-1,
        )
        nc.gpsimd.dma_start(out=wp_sb[:], in_=wp)
        nc.gpsimd.dma_start(out=wp2_sb[:], in_=wp[HD:128, :])

        # ---- merged pipeline over (batch, t-block) ----
        PW = min(512, d)
        NIB = d // PW
        blocks = [(bb, j) for bb in range(b) for j in range(NJ)]
        with ExitStack() as body:
            xpool = body.enter_context(tc.tile_pool(name="xpool", bufs=3))
            vtpool = body.enter_context(tc.tile_pool(name="vtpool", bufs=2))
            npool = body.enter_context(tc.tile_pool(name="npool", bufs=10))
            zpool = body.enter_context(tc.tile_pool(name="zpool", bufs=4))
            tmpool = body.enter_context(tc.tile_pool(name="tmpool", bufs=4))
            opool = body.enter_context(tc.tile_pool(name="opool", bufs=4))
            # PSUM budget (8 banks): qkv 2 + scores/bcast 2 + av 2 + tr/proj 2
            ps_qkv = body.enter_context(tc.tile_pool(name="ps_qkv", bufs=2, space="PSUM"))
            ps_s = body.enter_context(tc.tile_pool(name="ps_s", bufs=2, space="PSUM"))
            ps_av = body.enter_context(tc.tile_pool(name="ps_av", bufs=2, space="PSUM"))
            ps_tp = body.enter_context(tc.tile_pool(name="ps_tp", bufs=2, space="PSUM"))

            def gen_qkv(bidx):
                """QKV + v-transpose for block bidx, yielded in small units."""
                if bidx >= len(blocks):
                    return
                bb, j = blocks[bidx]
                col0 = bb * t + j * TBLK
                tsl = slice(col0, col0 + TBLK)
                # split loads (first is a single ktile): the first QKV
                # matmul waits on 0.25MB, and transfers pipeline with the
                # accumulation chains
                KQ = [max(1, KT // 4)] * 4
                KQ[-1] = KT - sum(KQ[:-1])
                xh, xt, k0 = [], [], 0
                for q, kq in enumerate(KQ):
                    xq = xpool.tile([128, kq, TBLK], mdt, tag=f"xh{q}",
                                    name=f"xh_{bb}_{j}_{q}")
                    nc.sync.dma_start(
                        out=xq[:],
                        in_=xT[k0 * 128:(k0 + kq) * 128, tsl]
                            .rearrange("(kt p) m -> p kt m", p=128),
                    )
                    xt += [xq[:, i, :] for i in range(kq)]
                    k0 += kq
                yield
                for w_sb, dst, nm in ((wq_sb, qT_sb, "q"), (wk_sb, kT_sb, "k")):
                    ps = ps_qkv.tile([128, TBLK], f32, tag="ps_qkv",
                                     name=f"ps{nm}_{bb}_{j}")
                    for kt in range(KT):
                        nc.tensor.matmul(ps[:], w_sb[kt], xt[kt],
                                         start=(kt == 0), stop=(kt == KT - 1))
                        if kt % 2 == 1:
                            yield
                    nc.vector.tensor_copy(dst[:, tsl], ps[:])
                    yield
                ps = ps_qkv.tile([128, TBLK], f32, tag="ps_qkv", name=f"psv_{bb}_{j}")
                for kt in range(KT):
                    nc.tensor.matmul(ps[:], wv_sb[kt], xt[kt],
                                     start=(kt == 0), stop=(kt == KT - 1))
                    if kt % 2 == 1:
                        yield
                vt = vtpool.tile([128, TBLK], mdt, tag="vt", name=f"vt_{bb}_{j}")
                nc.vector.tensor_copy(vt[:], ps[:])
                yield
                for s4 in range(SPT):
                    sb_idx = (col0 // 128) + s4
                    pt = ps_tp.tile([128, 128], f32, tag="ps_tp",
                                    name=f"ptr_{bb}_{j}_{s4}")
                    pout = pt[:].bitcast(mdt) if mdt != f32 else pt[:]
                    nc.tensor.transpose(pout, vt[:, s4 * 128:(s4 + 1) * 128], ident[:])
                    nc.vector.tensor_copy(
                        vaug[:, sb_idx, :].rearrange(
                            "p (g c) -> p g c", g=HPC)[:, :, 0:HD],
                        pt[:].rearrange("p (g c) -> p g c", g=HPC),
                    )
                    yield

            def gen_proj(blk):
                """Projection of a finished block's outT columns -> out_p.
                One wide store per 128-row chunk keeps the serial HWDGE
                generator and SP dispatch stream short."""
                bb, j = blk
                col0 = bb * t + j * TBLK
                u = 0
                for tl in range(TBLK // 128):
                    tt = col0 // 128 + tl
                    ot = opool.tile([128, d], bf16, tag="ot",
                                    name=f"ot_{bb}_{j}_{tl}")
                    for ib in range(NIB):
                        ps = ps_tp.tile([128, PW], f32, tag="ps_tp",
                                        name=f"psp_{bb}_{j}_{tl}_{ib}")
                        nc.tensor.matmul(ps[:], outT_sb[:, tt * 128:(tt + 1) * 128],
                                         wp_sb[:, ib * PW:(ib + 1) * PW],
                                         start=True, stop=True)
                        # GPSIMD cannot read PSUM on HW: PSUM drains are DVE's
                        nc.vector.tensor_copy(ot[:, ib * PW:(ib + 1) * PW], ps[:])
                        u += 1
                        yield
                    nc.sync.dma_start(out=out_p[tt * 128:(tt + 1) * 128, :],
                                      in_=ot[:])

            def advance(g):
                if g is None:
                    return False
                try:
                    next(g)
                    return True
                except StopIteration:
                    return False

            def emit_norm(bb, j, h, cl, cr, avs, bc_pool, h1_dst=None):
                """Normalize avs[h] columns [cl:cr) into outT: 1/Z on DVE,
                broadcast across partitions by a K=1 matmul, multiplied on
                DVE. h1 is shifted to outT partitions 64..128 by an
                SBUF->SBUF DMA, unless h1_dst is given (tail: stays at
                partitions 0:64 for the split-K projection)."""
                w = cr - cl
                col0 = bb * t + j * TBLK
                rr = zpool.tile([65, TBLK], mdt, tag="rr",
                                name=f"rr_{bb}_{j}_{h}_{cl}")
                with nc.allow_low_precision(reason="tf32 softmax denom"):
                    nc.vector.reciprocal(rr[64:65, 0:w], avs[h][64:65, cl:cr])
                bc = bc_pool.tile([HD, TBLK], f32, tag=bc_pool.name,
                                  name=f"bc_{bb}_{j}_{h}_{cl}")
                nc.tensor.matmul(bc[:, 0:w], ones1[64:65, :], rr[64:65, 0:w],
                                 start=True, stop=True)
                bcs = tmpool.tile([HD, TBLK], f32, tag="bcs",
                                  name=f"bcs_{bb}_{j}_{h}_{cl}")
                if h == 0:
                    nc.scalar.copy(bcs[:, 0:w], bc[:, 0:w])
                else:
                    nc.vector.tensor_copy(bcs[:, 0:w], bc[:, 0:w])
                if h == 0:
                    nc.vector.tensor_mul(outT_sb[0:HD, col0 + cl:col0 + cr],
                                         avs[h][0:HD, cl:cr], bcs[:, 0:w])
                elif h1_dst is not None:
                    nc.vector.tensor_mul(h1_dst[:, cl:cr], avs[h][0:HD, cl:cr],
                                         bcs[:, 0:w])
                else:
                    tmp = tmpool.tile([HD, TBLK], mdt, tag="tmp",
                                      name=f"tm_{bb}_{j}_{cl}")
                    nc.vector.tensor_mul(tmp[:, 0:w], avs[h][0:HD, cl:cr],
                                         bcs[:, 0:w])
                    nc.sync.dma_start(
                        out=outT_sb[h * HD:(h + 1) * HD, col0 + cl:col0 + cr],
                        in_=tmp[:, 0:w])

            projq = []           # deferred projection units (global backlog)
            projq_n = [0]        # unit count in backlog

            def adv_projq(k=1):
                done = 0
                while projq and done < k:
                    if advance(projq[0]):
                        projq_n[0] -= 1
                        done += 1
                    else:
                        projq.pop(0)
                return done

            def emit_attn(bb, j, feeds, steps_after, tail=False):
                """Attention for block (bb, j); feeds = [(gen, n_units)] of
                QKV work to interleave between i-steps. Projection units come
                from the global backlog at a rate that drains it over the
                remaining non-tail i-steps (late thin blocks get the most).
                tail=True: normalize + project each finished 128-column chunk
                inline (final block)."""
                col0 = bb * t + j * TBLK
                n_i = (j + 1) * SPT
                # prefetch unit 0 of each feed (dispatches next block's xt DMAs)
                gens = []
                for g, units in feeds:
                    if advance(g):
                        gens.append([g, (units - 1) / n_i, 0.0])
                denom = max(1, steps_after if not tail else n_i)
                prate = [projq_n[0] / denom, 0.0]

                def fill(k=1):
                    for _ in range(k):
                        got = False
                        for rec in gens:
                            if rec[0] is not None and rec[2] >= 1.0:
                                if advance(rec[0]):
                                    rec[2] -= 1.0
                                    got = True
                                else:
                                    rec[0] = None
                        if not got:
                            break
                    prate[1] += prate[0]
                    if prate[1] >= 1.0:
                        prate[1] -= adv_projq(int(prate[1]))

                avs = [ps_av.tile([65, TBLK], f32, tag="ps_av",
                                  name=f"av{h}_{bb}_{j}") for h in range(HPC)]

                def emit_av(i_, c0_, nh_pair):
                    sb_idx = (bb * t + i_ * 128) // 128
                    st, sp = (i_ == 0), (i_ == n_i - 1)
                    for h in range(HPC):
                        nc.tensor.matmul(avs[h][:, c0_:TBLK],
                                         vaug[:, sb_idx, h * 66:h * 66 + HD + 1],
                                         nh_pair[h][:, c0_:TBLK], start=st, stop=sp,
                                         skip_group_check=True)

                o1 = (tmpool.tile([HD, TBLK], mdt, tag="o1", name=f"o1_{bb}_{j}")
                      if tail else None)

                def tail_chunk(s4):
                    # columns [128*s4, 128*s4+128) are final: normalize and
                    # project them now so stores drain under remaining
                    # compute. h1 stays at partitions 0:64 (o1); the split-K
                    # projection (vs wp rows 64:128 staged at partitions
                    # 0:64) avoids the cross-partition staging DMA latency.
                    cl, cr = 128 * s4, 128 * s4 + 128
                    for h in range(HPC):
                        emit_norm(bb, j, h, cl, cr, avs, ps_qkv, h1_dst=o1)
                    tt = col0 // 128 + s4
                    ot = opool.tile([128, d], bf16, tag="ot",
                                    name=f"ot_{bb}_{j}_t{s4}")
                    for ib in range(NIB):
                        ps = ps_tp.tile([128, PW], f32, tag="ps_tp",
                                        name=f"psp_{bb}_{j}_{s4}_{ib}")
                        nc.tensor.matmul(ps[:], outT_sb[0:HD, tt * 128:(tt + 1) * 128],
                                         wp_sb[0:HD, ib * PW:(ib + 1) * PW],
                                         start=True, stop=False)
                        nc.tensor.matmul(ps[:], o1[:, cl:cr],
                                         wp2_sb[:, ib * PW:(ib + 1) * PW],
                                         start=False, stop=True)
                        # ACT is idle once the last block's exps are done
                        nc.scalar.copy(ot[:, ib * PW:(ib + 1) * PW], ps[:])
                    nc.sync.dma_start(out=out_p[tt * 128:(tt + 1) * 128, :],
                                      in_=ot[:])

                prev = None
                for i in range(n_i):
                    ssl = slice(bb * t + i * 128, bb * t + i * 128 + 128)
                    dd = 128 * i - TBLK * j
                    c0 = max(dd, 0)
                    nh_pair = []
                    for h in range(HPC):
                        hp = slice(h * HD, (h + 1) * HD)
                        ps = ps_s.tile([128, TBLK], f32, tag="ps_s",
                                       name=f"pss_{bb}_{j}_{i}_{h}")
                        nc.tensor.matmul(ps[:, c0:TBLK], kT_sb[hp, ssl],
                                         qT_sb[hp, col0 + c0:col0 + TBLK],
                                         start=True, stop=True)
                        nh = npool.tile([128, TBLK], mdt, tag="nh",
                                        name=f"nh_{bb}_{j}_{i}_{h}")
                        nc.scalar.activation(
                            nh[:, c0:TBLK], ps[:, c0:TBLK],
                            mybir.ActivationFunctionType.Exp, scale=0.125)
                        if dd >= 0:
                            nc.gpsimd.tensor_mul(
                                nh[:, dd:dd + 128], nh[:, dd:dd + 128], mask[:])
                        nh_pair.append(nh)
                    # attn@v lags one i-step so exp (ACT) hides under PE
                    if prev is not None:
                        emit_av(*prev)
                        if tail and prev[0] >= SPT * j:
                            tail_chunk(prev[0] - SPT * j)
                    prev = (i, c0, nh_pair)
                    for rec in gens:
                        rec[2] += rec[1]
                    fill(4)
                emit_av(*prev)
                if tail:
                    if prev[0] >= SPT * j:
                        tail_chunk(prev[0] - SPT * j)
                    for rec in gens:
                        while advance(rec[0]):
                            pass
                    return

                for rec in gens:
                    rec[2] += 1.0
                fill(2)
                for h in range(HPC):
                    emit_norm(bb, j, h, 0, TBLK, avs, ps_tp)
                    fill(1)
                # drain leftover QKV (it has a deadline); proj stays backlogged
                for rec in gens:
                    while advance(rec[0]):
                        pass
                projq.append(gen_proj((bb, j)))
                projq_n[0] += (TBLK // 128) * NIB

            # software pipeline: QKV runs ahead (natural block order) and
            # projection one behind. Attention order rotates the last batch
            # so the FINAL block is a small j=0 one: its short tail (norm +
            # split-K projection per chunk) is all that drains at the end.
            order = []
            for bbb in range(b):
                js = list(range(NJ))
                if bbb == b - 1 and NJ > 1:
                    js = js[1:] + [0]
                order += [(bbb, jj) for jj in js]
            for _ in gen_qkv(0):
                pass
            ptr = 1
            n_is = [(jj + 1) * SPT for (_, jj) in order]
            for p, blk in enumerate(order):
                feeds = []
                if p + 1 < len(order):
                    nb, njj = order[p + 1]
                    target = nb * NJ + njj + 1
                else:
                    target = ptr
                while ptr < target:
                    feeds.append((gen_qkv(ptr), 20))
                    ptr += 1
                # i-steps remaining in non-tail blocks from here on
                steps_after = sum(n_is[p:len(order) - 1])
                emit_attn(*blk, feeds, steps_after,
                          tail=(p == len(order) - 1))
            while adv_projq(1):
                pass

    nc.compile()
    return nc


def _get_nc(b=B, t=T, d=D, cfg="tf32"):
    key = (b, t, d, cfg)
    if key not in _CACHE:
        _CACHE[key] = _build(b, t, d, cfg)
    return _CACHE[key]


def _prepare_in_maps(x, Wq, Wk, Wv, Wp, b, t, d, n_heads):
    bt = b * t
    xT = np.ascontiguousarray(x.reshape(bt, d).T.astype(np.float32))
    in_maps = []
    for c in range(NCORES):
        h0 = c * HPC
        wq_c = np.ascontiguousarray(Wq[h0:h0 + HPC].reshape(CH, d).T.astype(np.float32))
        wk_c = np.ascontiguousarray(Wk[h0:h0 + HPC].reshape(CH, d).T.astype(np.float32))
        wv_c = np.ascontiguousarray(Wv[h0:h0 + HPC].reshape(CH, d).T.astype(np.float32))
        wp_c = np.ascontiguousarray(Wp[:, c * CH:(c + 1) * CH].T.astype(np.float32))
        in_maps.append({"xT": xT, "wq": wq_c, "wk": wk_c, "wv": wv_c, "wp": wp_c})
    return in_maps


def _run(x, Wq, Wk, Wv, Wp, bp, b, t, d, cfg, trace=False):
    from concourse.bass_utils import run_bass_kernel_spmd
    nc = _get_nc(b, t, d, cfg)
    in_maps = _prepare_in_maps(x, Wq, Wk, Wv, Wp, b, t, d, H)
    res = run_bass_kernel_spmd(nc, in_maps, core_ids=list(range(NCORES)), trace=trace)
    acc = np.zeros((b * t, d), dtype=np.float64)
    for r in res.results:
        acc += r["out_p"].astype(np.float64)
    out = (acc + np.asarray(bp, dtype=np.float64)).astype(np.float32)
    return out.reshape(b, t, d), res


KERNEL_CFG = "tf32"


def kernel(x, Wq, Wk, Wv, Wp, bp):
    out, _ = _run(np.asarray(x), np.asarray(Wq), np.asarray(Wk), np.asarray(Wv),
                  np.asarray(Wp), np.asarray(bp), B, T, D, KERNEL_CFG, trace=False)
    return out


# revision 49
# speedup vs baseline: 1.0056x; 1.0056x over previous
"""Multi-head attention Trainium2 kernel (8 NeuronCores, tensor-parallel over heads).

Strategy:
  - 16 heads / 8 cores = 2 heads per core. x is replicated; Wq/Wk/Wv sharded by
    head; Wp row-sharded (contraction dim). Each core computes a partial
    projection output [B*T, D]; the host sums the 8 partials (+bias).
  - On chip, all contractions need the contracted dim on SBUF partitions, so the
    host passes xT = x.reshape(BT, D).T and per-core transposed weight slices.
  - qT/kT are computed packed [128 = 2 heads x 64, BT]. Scores are computed
    transposed (s on partitions, t on free) so softmax normalization can ride
    the attn@v matmul: lhsT = [v_h | ones] gives unnormalized out^T plus the
    softmax denominator Z as an extra row. Head 0 lands at PSUM partitions
    0..64 (Z at 64), head 1 at partitions 63..128 (Z at 63, ones column FIRST
    in its vaug slice) so both normalization multiplies are partition-aligned
    with their outT rows -- no cross-partition staging DMA.
  - Causality: only lower-triangular blocks are computed; matmuls straddling
    the diagonal are column-trimmed to the causal region and share one
    [128,128] staircase mask (applied multiplicatively after exp).
  - cfg "tf32": every matmul operand tile/DRAM tensor is float32r (TF32-like,
    1 cycle/row vs 4 for fp32 when N>=256). The BIR verifier requires f32r
    matmul inputs be *produced* as f32r, so DRAM inputs are declared f32r and
    every on-chip producer (DVE/ACT/Pool copies, exp, muls) writes f32r.
  - Emission is interleaved: engines execute their streams in program order,
    so QKV (next block) and projection (previous block) PE work is emitted in
    small units between attention i-steps, filling PE stalls that the
    scores->exp->attn@v dependency chain would otherwise expose.
"""

import numpy as np

B, T, D, H, HD = 2, 2048, 1024, 16, 64
NCORES = 8
HPC = H // NCORES          # heads per core = 2
CH = HPC * HD              # channels per core = 128
BT = B * T

_CACHE = {}


def _build(b, t, d, cfg):
    """Build + compile the per-core Bass program."""
    import concourse.tile as tile
    from concourse import bacc, mybir
    from concourse.masks import make_identity
    from contextlib import ExitStack

    f32 = mybir.dt.float32
    mdt = mybir.dt.float32r if cfg == "tf32" else f32

    bt = b * t
    KT = d // 128            # k-tiles over the model dim
    TBLK = min(512, t)       # t-block width for scores/attn
    NJ = t // TBLK           # t-blocks per batch
    NSB = bt // 128          # 128-row s-blocks over B*T
    SPT = TBLK // 128        # s-blocks per t-block

    nc = bacc.Bacc("TRN2", target_bir_lowering=False, debug=False)

    xT = nc.dram_tensor("xT", [d, bt], mdt, kind="ExternalInput").ap()
    wq = nc.dram_tensor("wq", [d, CH], mdt, kind="ExternalInput").ap()
    wk = nc.dram_tensor("wk", [d, CH], mdt, kind="ExternalInput").ap()
    wv = nc.dram_tensor("wv", [d, CH], mdt, kind="ExternalInput").ap()
    wp = nc.dram_tensor("wp", [CH, d], mdt, kind="ExternalInput").ap()
    bf16 = mybir.dt.bfloat16
    out_p = nc.dram_tensor("out_p", [bt, d], bf16, kind="ExternalOutput").ap()

    with tile.TileContext(nc) as tc, ExitStack() as top:
        persist = top.enter_context(tc.tile_pool(name="persist", bufs=1))

        # ---- persistent tiles ----
        qT_sb = persist.tile([128, bt], mdt, tag="qT")
        kT_sb = persist.tile([128, bt], mdt, tag="kT")
        # per 128-row s-block: [v_h0 | 1 | pad | v_h1 | 1 | pad]
        vaug = persist.tile([128, NSB, 66 * HPC], mdt, tag="vaug")
        outT_sb = persist.tile([128, bt], mdt, tag="outT")
        wq_3d = persist.tile([128, KT, CH], mdt, tag="wq")
        wk_3d = persist.tile([128, KT, CH], mdt, tag="wk")
        wv_3d = persist.tile([128, KT, CH], mdt, tag="wv")
        wq_sb = [wq_3d[:, kt, :] for kt in range(KT)]
        wk_sb = [wk_3d[:, kt, :] for kt in range(KT)]
        wv_sb = [wv_3d[:, kt, :] for kt in range(KT)]
        wp_sb = persist.tile([128, d], mdt, tag="wp")
        # copy of wp rows 64:128 at partitions 0:64 for the tail's split-K
        # projection (h1 output stays at partitions 0:64 there)
        wp2_sb = persist.tile([HD, d], mdt, tag="wp2")
        ident_f32 = persist.tile([128, 128], f32, tag="ident_f32")
        mask = persist.tile([128, 128], f32, tag="mask")
        ones_f32 = persist.tile([128, max(NSB, HD)], f32, tag="ones_f32")
        ones1 = persist.tile([65, HD], mdt, tag="ones1")

        # weight loads spread across dispatch queues so SP stays free for xt
        # wq first on SP so its transfer beats the xt loads to the DMA
        # engines; wk/wv/wp ride Pool's SWDGE (cheap dispatch)
        nc.sync.dma_start(
            out=wq_3d[:], in_=wq.rearrange("(kt p) m -> p kt m", p=128))
        nc.gpsimd.dma_start(
            out=wk_3d[:], in_=wk.rearrange("(kt p) m -> p kt m", p=128))
        nc.gpsimd.dma_start(
            out=wv_3d[:], in_=wv.rearrange("(kt p) m -> p kt m", p=128))

        make_identity(nc, ident_f32[:])
        if mdt != f32:
            ident = persist.tile([128, 128], mdt, tag="ident")
            nc.scalar.copy(ident[:], ident_f32[:])
        else:
            ident = ident_f32
        nc.gpsimd.memset(ones_f32[:], 1.0)
        # ones row for the 1/Z broadcast matmuls (K=1 lhsT at partition 64)
        nc.scalar.copy(ones1[64:65, :], ones_f32[64:65, 0:HD])
        # ones columns of vaug (softmax denominator rides the attn@v matmul)
        for g in range(HPC):
            nc.scalar.copy(
                vaug[:, :, g * 66 + 64:g * 66 + 65],
                ones_f32[:, 0:NSB].rearrange("p (n o) -> p n o", o=1),
            )
        # staircase mask: keep (p <= c), i.e. upper-triangular incl. diagonal
        nc.gpsimd.memset(mask[:], 1.0)
        nc.gpsimd.affine_select(
            out=mask[:], in_=mask[:],
            compare_op=mybir.AluOpType.is_ge,
            fill=0.0, base=0,
            # iota = -p + c ; keep when >= 0
            pattern=[[1, 128]], channel_multiplier=-1,
        )
        nc.gpsimd.dma_start(out=wp_sb[:], in_=wp)
        nc.gpsimd.dma_start(out=wp2_sb[:], in_=wp[HD:128, :])

        # ---- merged pipeline over (batch, t-block) ----
        PW = min(512, d)
        NIB = d // PW
        blocks = [(bb, j) for bb in range(b) for j in range(NJ)]
        with ExitStack() as body:
            xpool = body.enter_context(tc.tile_pool(name="xpool", bufs=3))
            vtpool = body.enter_context(tc.tile_pool(name="vtpool", bufs=2))
            npool = body.enter_context(tc.tile_pool(name="npool", bufs=10))
            zpool = body.enter_context(tc.tile_pool(name="zpool", bufs=4))
            tmpool = body.enter_context(tc.tile_pool(name="tmpool", bufs=4))
            opool = body.enter_context(tc.tile_pool(name="opool", bufs=4))
            # PSUM budget (8 banks): qkv 2 + scores/bcast 2 + av 2 + tr/proj 2
            ps_qkv = body.enter_context(tc.tile_pool(name="ps_qkv", bufs=2, space="PSUM"))
            ps_s = body.enter_context(tc.tile_pool(name="ps_s", bufs=2, space="PSUM"))
            ps_av = body.enter_context(tc.tile_pool(name="ps_av", bufs=2, space="PSUM"))
            ps_tp = body.enter_context(tc.tile_pool(name="ps_tp", bufs=2, space="PSUM"))

            def gen_qkv(bidx):
                """QKV + v-transpose for block bidx, yielded in small units."""
                if bidx >= len(blocks):
                    return
                bb, j = blocks[bidx]
                col0 = bb * t + j * TBLK
                tsl = slice(col0, col0 + TBLK)
                # split loads (first is a single ktile): the first QKV
                # matmul waits on 0.25MB, and transfers pipeline with the
                # accumulation chains
                KQ = [max(1, KT // 4)] * 4
                KQ[-1] = KT - sum(KQ[:-1])
                xh, xt, k0 = [], [], 0
                for q, kq in enumerate(KQ):
                    xq = xpool.tile([128, kq, TBLK], mdt, tag=f"xh{q}",
                                    name=f"xh_{bb}_{j}_{q}")
                    nc.sync.dma_start(
                        out=xq[:],
                        in_=xT[k0 * 128:(k0 + kq) * 128, tsl]
                            .rearrange("(kt p) m -> p kt m", p=128),
                    )
                    xt += [xq[:, i, :] for i in range(kq)]
                    k0 += kq
                yield
                for w_sb, dst, nm in ((wq_sb, qT_sb, "q"), (wk_sb, kT_sb, "k")):
                    ps = ps_qkv.tile([128, TBLK], f32, tag="ps_qkv",
                                     name=f"ps{nm}_{bb}_{j}")
                    for kt in range(KT):
                        nc.tensor.matmul(ps[:], w_sb[kt], xt[kt],
                                         start=(kt == 0), stop=(kt == KT - 1))
                        yield
                    nc.vector.tensor_copy(dst[:, tsl], ps[:])
                    yield
                ps = ps_qkv.tile([128, TBLK], f32, tag="ps_qkv", name=f"psv_{bb}_{j}")
                for kt in range(KT):
                    nc.tensor.matmul(ps[:], wv_sb[kt], xt[kt],
                                     start=(kt == 0), stop=(kt == KT - 1))
                    yield
                vt = vtpool.tile([128, TBLK], mdt, tag="vt", name=f"vt_{bb}_{j}")
                nc.vector.tensor_copy(vt[:], ps[:])
                yield
                for s4 in range(SPT):
                    sb_idx = (col0 // 128) + s4
                    pt = ps_tp.tile([128, 128], f32, tag="ps_tp",
                                    name=f"ptr_{bb}_{j}_{s4}")
                    pout = pt[:].bitcast(mdt) if mdt != f32 else pt[:]
                    nc.tensor.transpose(pout, vt[:, s4 * 128:(s4 + 1) * 128], ident[:])
                    nc.vector.tensor_copy(
                        vaug[:, sb_idx, :].rearrange(
                            "p (g c) -> p g c", g=HPC)[:, :, 0:HD],
                        pt[:].rearrange("p (g c) -> p g c", g=HPC),
                    )
                    yield

            def gen_proj(blk):
                """Projection of a finished block's outT columns -> out_p.
                One wide store per 128-row chunk keeps the serial HWDGE
                generator and SP dispatch stream short."""
                bb, j = blk
                col0 = bb * t + j * TBLK
                u = 0
                for tl in range(TBLK // 128):
                    tt = col0 // 128 + tl
                    ot = opool.tile([128, d], bf16, tag="ot",
                                    name=f"ot_{bb}_{j}_{tl}")
                    for ib in range(NIB):
                        ps = ps_tp.tile([128, PW], f32, tag="ps_tp",
                                        name=f"psp_{bb}_{j}_{tl}_{ib}")
                        nc.tensor.matmul(ps[:], outT_sb[:, tt * 128:(tt + 1) * 128],
                                         wp_sb[:, ib * PW:(ib + 1) * PW],
                                         start=True, stop=True)
                        # GPSIMD cannot read PSUM on HW: PSUM drains are DVE's
                        nc.vector.tensor_copy(ot[:, ib * PW:(ib + 1) * PW], ps[:])
                        u += 1
                        yield
                    nc.sync.dma_start(out=out_p[tt * 128:(tt + 1) * 128, :],
                                      in_=ot[:])

            def advance(g):
                if g is None:
                    return False
                try:
                    next(g)
                    return True
                except StopIteration:
                    return False

            def emit_norm(bb, j, h, cl, cr, avs, bc_pool, h1_dst=None):
                """Normalize avs[h] columns [cl:cr) into outT: 1/Z on DVE,
                broadcast across partitions by a K=1 matmul, multiplied on
                DVE. h1 is shifted to outT partitions 64..128 by an
                SBUF->SBUF DMA, unless h1_dst is given (tail: stays at
                partitions 0:64 for the split-K projection)."""
                w = cr - cl
                col0 = bb * t + j * TBLK
                rr = zpool.tile([65, TBLK], mdt, tag="rr",
                                name=f"rr_{bb}_{j}_{h}_{cl}")
                with nc.allow_low_precision(reason="tf32 softmax denom"):
                    nc.vector.reciprocal(rr[64:65, 0:w], avs[h][64:65, cl:cr])
                bc = bc_pool.tile([HD, TBLK], f32, tag=bc_pool.name,
                                  name=f"bc_{bb}_{j}_{h}_{cl}")
                nc.tensor.matmul(bc[:, 0:w], ones1[64:65, :], rr[64:65, 0:w],
                                 start=True, stop=True)
                bcs = tmpool.tile([HD, TBLK], f32, tag="bcs",
                                  name=f"bcs_{bb}_{j}_{h}_{cl}")
                if h == 0:
                    nc.scalar.copy(bcs[:, 0:w], bc[:, 0:w])
                else:
                    nc.vector.tensor_copy(bcs[:, 0:w], bc[:, 0:w])
                if h == 0:
                    nc.vector.tensor_mul(outT_sb[0:HD, col0 + cl:col0 + cr],
                                         avs[h][0:HD, cl:cr], bcs[:, 0:w])
                elif h1_dst is not None:
                    nc.vector.tensor_mul(h1_dst[:, cl:cr], avs[h][0:HD, cl:cr],
                                         bcs[:, 0:w])
                else:
                    tmp = tmpool.tile([HD, TBLK], mdt, tag="tmp",
                                      name=f"tm_{bb}_{j}_{cl}")
                    nc.vector.tensor_mul(tmp[:, 0:w], avs[h][0:HD, cl:cr],
                                         bcs[:, 0:w])
                    nc.sync.dma_start(
                        out=outT_sb[h * HD:(h + 1) * HD, col0 + cl:col0 + cr],
                        in_=tmp[:, 0:w])

            projq = []           # deferred projection units (global backlog)
            projq_n = [0]        # unit count in backlog

            def adv_projq(k=1):
                done = 0
                while projq and done < k:
                    if advance(projq[0]):
                        projq_n[0] -= 1
                        done += 1
                    else:
                        projq.pop(0)
                return done

            def emit_attn(bb, j, feeds, steps_after, tail=False):
                """Attention for block (bb, j); feeds = [(gen, n_units)] of
                QKV work to interleave between i-steps. Projection units come
                from the global backlog at a rate that drains it over the
                remaining non-tail i-steps (late thin blocks get the most).
                tail=True: normalize + project each finished 128-column chunk
                inline (final block)."""
                col0 = bb * t + j * TBLK
                n_i = (j + 1) * SPT
                # prefetch unit 0 of each feed (dispatches next block's xt DMAs)
                gens = []
                for g, units in feeds:
                    if advance(g):
                        gens.append([g, (units - 1) / n_i, 0.0])
                denom = max(1, steps_after if not tail else n_i)
                prate = [projq_n[0] / denom, 0.0]

                def fill(k=1):
                    for _ in range(k):
                        got = False
                        for rec in gens:
                            if rec[0] is not None and rec[2] >= 1.0:
                                if advance(rec[0]):
                                    rec[2] -= 1.0
                                    got = True
                                else:
                                    rec[0] = None
                        if not got:
                            break
                    prate[1] += prate[0]
                    if prate[1] >= 1.0:
                        prate[1] -= adv_projq(int(prate[1]))

                avs = [ps_av.tile([65, TBLK], f32, tag="ps_av",
                                  name=f"av{h}_{bb}_{j}") for h in range(HPC)]

                def emit_av(i_, c0_, nh_pair):
                    sb_idx = (bb * t + i_ * 128) // 128
                    st, sp = (i_ == 0), (i_ == n_i - 1)
                    for h in range(HPC):
                        nc.tensor.matmul(avs[h][:, c0_:TBLK],
                                         vaug[:, sb_idx, h * 66:h * 66 + HD + 1],
                                         nh_pair[h][:, c0_:TBLK], start=st, stop=sp,
                                         skip_group_check=True)

                o1 = (tmpool.tile([HD, TBLK], mdt, tag="o1", name=f"o1_{bb}_{j}")
                      if tail else None)

                def tail_chunk(s4):
                    # columns [128*s4, 128*s4+128) are final: normalize and
                    # project them now so stores drain under remaining
                    # compute. h1 stays at partitions 0:64 (o1); the split-K
                    # projection (vs wp rows 64:128 staged at partitions
                    # 0:64) avoids the cross-partition staging DMA latency.
                    cl, cr = 128 * s4, 128 * s4 + 128
                    for h in range(HPC):
                        emit_norm(bb, j, h, cl, cr, avs, ps_qkv, h1_dst=o1)
                    tt = col0 // 128 + s4
                    ot = opool.tile([128, d], bf16, tag="ot",
                                    name=f"ot_{bb}_{j}_t{s4}")
                    for ib in range(NIB):
                        ps = ps_tp.tile([128, PW], f32, tag="ps_tp",
                                        name=f"psp_{bb}_{j}_{s4}_{ib}")
                        nc.tensor.matmul(ps[:], outT_sb[0:HD, tt * 128:(tt + 1) * 128],
                                         wp_sb[0:HD, ib * PW:(ib + 1) * PW],
                                         start=True, stop=False)
                        nc.tensor.matmul(ps[:], o1[:, cl:cr],
                                         wp2_sb[:, ib * PW:(ib + 1) * PW],
                                         start=False, stop=True)
                        # ACT is idle once the last block's exps are done;
                        # store each half as soon as its copy lands so the
                        # final drain is one half-store, not copy+copy+store
                        nc.scalar.copy(ot[:, ib * PW:(ib + 1) * PW], ps[:])
                        nc.sync.dma_start(
                            out=out_p[tt * 128:(tt + 1) * 128,
                                      ib * PW:(ib + 1) * PW],
                            in_=ot[:, ib * PW:(ib + 1) * PW])

                prev = None
                for i in range(n_i):
                    ssl = slice(bb * t + i * 128, bb * t + i * 128 + 128)
                    dd = 128 * i - TBLK * j
                    c0 = max(dd, 0)
                    nh_pair = []
                    for h in range(HPC):
                        hp = slice(h * HD, (h + 1) * HD)
                        ps = ps_s.tile([128, TBLK], f32, tag="ps_s",
                                       name=f"pss_{bb}_{j}_{i}_{h}")
                        nc.tensor.matmul(ps[:, c0:TBLK], kT_sb[hp, ssl],
                                         qT_sb[hp, col0 + c0:col0 + TBLK],
                                         start=True, stop=True)
                        nh = npool.tile([128, TBLK], mdt, tag="nh",
                                        name=f"nh_{bb}_{j}_{i}_{h}")
                        nc.scalar.activation(
                            nh[:, c0:TBLK], ps[:, c0:TBLK],
                            mybir.ActivationFunctionType.Exp, scale=0.125)
                        if dd >= 0:
                            nc.gpsimd.tensor_mul(
                                nh[:, dd:dd + 128], nh[:, dd:dd + 128], mask[:])
                        nh_pair.append(nh)
                    # attn@v lags one i-step so exp (ACT) hides under PE
                    if prev is not None:
                        emit_av(*prev)
                        if tail and prev[0] >= SPT * j:
                            tail_chunk(prev[0] - SPT * j)
                    prev = (i, c0, nh_pair)
                    for rec in gens:
                        rec[2] += rec[1]
                    fill(8)
                emit_av(*prev)
                if tail:
                    if prev[0] >= SPT * j:
                        tail_chunk(prev[0] - SPT * j)
                    for rec in gens:
                        while advance(rec[0]):
                            pass
                    return

                for rec in gens:
                    rec[2] += 1.0
                fill(2)
                for h in range(HPC):
                    emit_norm(bb, j, h, 0, TBLK, avs, ps_tp)
                    fill(1)
                # drain leftover QKV (it has a deadline); proj stays backlogged
                for rec in gens:
                    while advance(rec[0]):
                        pass
                projq.append(gen_proj((bb, j)))
                projq_n[0] += (TBLK // 128) * NIB

            # software pipeline: QKV runs ahead (natural block order) and
            # projection one behind. Attention order rotates the last batch
            # so the FINAL block is a small j=0 one: its short tail (norm +
            # split-K projection per chunk) is all that drains at the end.
            order = []
            for bbb in range(b):
                js = list(range(NJ))
                if bbb == b - 1 and NJ > 1:
                    js = js[1:] + [0]
                order += [(bbb, jj) for jj in js]
            for _ in gen_qkv(0):
                pass
            ptr = 1
            n_is = [(jj + 1) * SPT for (_, jj) in order]
            for p, blk in enumerate(order):
                feeds = []
                if p + 1 < len(order):
                    nb, njj = order[p + 1]
                    target = nb * NJ + njj + 1
                else:
                    target = ptr
                while ptr < target:
                    feeds.append((gen_qkv(ptr), 32))
                    ptr += 1
                # i-steps remaining in non-tail blocks from here on
                steps_after = sum(n_is[p:len(order) - 1])
                emit_attn(*blk, feeds, steps_after,
                          tail=(p == len(order) - 1))
            while adv_projq(1):
                pass

    nc.compile()
    return nc


def _get_nc(b=B, t=T, d=D, cfg="tf32"):
    key = (b, t, d, cfg)
    if key not in _CACHE:
        _CACHE[key] = _build(b, t, d, cfg)
    return _CACHE[key]


def _prepare_in_maps(x, Wq, Wk, Wv, Wp, b, t, d, n_heads):
    bt = b * t
    xT = np.ascontiguousarray(x.reshape(bt, d).T.astype(np.float32))
    in_maps = []
    for c in range(NCORES):
        h0 = c * HPC
        wq_c = np.ascontiguousarray(Wq[h0:h0 + HPC].reshape(CH, d).T.astype(np.float32))
        wk_c = np.ascontiguousarray(Wk[h0:h0 + HPC].reshape(CH, d).T.astype(np.float32))
        wv_c = np.ascontiguousarray(Wv[h0:h0 + HPC].reshape(CH, d).T.astype(np.float32))
        wp_c = np.ascontiguousarray(Wp[:, c * CH:(c + 1) * CH].T.astype(np.float32))
        in_maps.append({"xT": xT, "wq": wq_c, "wk": wk_c, "wv": wv_c, "wp": wp_c})
    return in_maps


def _run(x, Wq, Wk, Wv, Wp, bp, b, t, d, cfg, trace=False):
    from concourse.bass_utils import run_bass_kernel_spmd
    nc = _get_nc(b, t, d, cfg)
    in_maps = _prepare_in_maps(x, Wq, Wk, Wv, Wp, b, t, d, H)
    res = run_bass_kernel_spmd(nc, in_maps, core_ids=list(range(NCORES)), trace=trace)
    acc = np.zeros((b * t, d), dtype=np.float64)
    for r in res.results:
        acc += r["out_p"].astype(np.float64)
    out = (acc + np.asarray(bp, dtype=np.float64)).astype(np.float32)
    return out.reshape(b, t, d), res


KERNEL_CFG = "tf32"


def kernel(x, Wq, Wk, Wv, Wp, bp):
    out, _ = _run(np.asarray(x), np.asarray(Wq), np.asarray(Wk), np.asarray(Wv),
                  np.asarray(Wp), np.asarray(bp), B, T, D, KERNEL_CFG, trace=False)
    return out
